# revision 55
# baseline (speedup 1.0000x reference)
"""NativeSparseAttention (fallback = full causal SDPA) Trainium2 kernel.

Sharding: 8 cores = 2 (batch) x 4 (kv head groups). Core (b, g) computes
q heads 4g..4g+3, kv head g, batch b, and a row-parallel partial of the
output projection; partials are summed on the host (the "all-reduce").

Schedule (v2): x/cos/sin stream in REVERSE column-block order (n=3..0) so
projections + head-0 scores start ~3us in; heads 1-3 stream scores 2
chunks ahead of P@V; head-3 sq carries the output projection; output is
fp16 partials. exp bottlenecks ACT (~58us floor), so everything else is
spread across PE/DVE/Pool.

Layouts on device (per core):
  xT    [1024, 2048] bf16   hidden_states[b].T (4 col-block DMAs per tile)
  qT    [256, 2048]  bf16   feature-major q (RoPE applied)
  kT    [64, 2048]   bf16   feature-major k (RoPE applied)
  v     [2048, 65]   bf16   token-major v with ones column (softmax denom)
  pT    [kv, sq]            scores transposed; exp(st/8) on ACT; diag mask
  attn  [sq, 65]     f32    PSUM accumulated over kv chunks; col 64 = denom
  ag    [2048, 256]  bf16   gated/normalized attn, token-major
  agT   [256, 2048]  bf16   PE-transposed for output projection
  outp  [2048, 1024] fp16   partial output
"""

import numpy as np
import ml_dtypes

import concourse.bass as bass
import concourse.mybir as mybir
import concourse.tile as tile
from concourse.bass_utils import run_bass_kernel_spmd
from concourse.masks import make_identity

FP32 = mybir.dt.float32
FP16 = mybir.dt.float16
BF16 = mybir.dt.bfloat16
FP8 = mybir.dt.float8e4
PM = mybir.MatmulPerfMode
AF = mybir.ActivationFunctionType
ALU = mybir.AluOpType


def _patch_tail_drain():
    """This container's walrus build allows only ONE semaphore wait per CTRL
    (Drain/NoOp) instruction, but Tile's kernel-tail drain attaches one wait
    per active queue/engine. Split the waits across preceding single-wait
    NOPs on the same engine (SP executes them in order, so semantics are
    unchanged)."""
    from bass_rust import ScopedClock

    if getattr(tile.TileContext, "_tail_drain_patched", False):
        return

    def _drain_and_barrier(self, tick_clock, wait_clock):
        nc = self.nc
        probe = nc.sync.nop(nofuse=True)
        wait_clock.add_sem_waits(
            probe.ins, ScopedClock({None: tick_clock.global_clock})
        )
        si = probe.ins.sync_info
        waits = list(si.on_wait) if si is not None else []
        if len(waits) > 1:
            si.on_wait = waits[:1]
            for w in waits[1:]:
                n2 = nc.sync.nop(nofuse=True)
                n2.ins.sync_info = mybir.SyncInfo(on_wait=[w], on_update=[])
        nc.sync.drain()
        nc.all_engine_barrier()
        popped = nc._tile_sem_poison_stack.pop()
        assert popped is self._sem_poison
        nc.clear_and_free_semaphores(list(self.sems.allocated().values()))
        nc.all_engine_barrier()

    tile.TileContext._drain_and_barrier = _drain_and_barrier
    tile.TileContext._tail_drain_patched = True


_patch_tail_drain()

PHASELOG = []

B = 2
S = 2048
HM = 1024
NH = 16
NKV = 4
D = 64
THETA = 10000.0
NCORES = 8

NCH = S // 128  # 16 sequence chunks of 128


def _split_multi_waits(nc: bass.Bass):
    """Walrus here allows a single semaphore wait per instruction; hoist
    extra waits onto same-engine NOPs placed immediately before (same
    sequencer, in-order => identical semantics)."""
    for f in nc.m.functions:
        for b in f.blocks:
            new = []
            changed = False
            for ins in b.instructions:
                si = ins.sync_info
                waits = list(si.on_wait) if si is not None else []
                if len(waits) > 1:
                    changed = True
                    for i, w in enumerate(waits[:-1]):
                        nop = mybir.InstNoOp(
                            name=f"{ins.name}-sw{i}",
                            sync_info=mybir.SyncInfo(on_wait=[w], on_update=[]),
                            bass_nofuse=True,
                            engine=ins.engine,
                        )
                        nc.register_instruction(nop, overwrite=True)
                        new.append(nop)
                    si.on_wait = waits[-1:]
                new.append(ins)
            if changed:
                b.instructions = new



def _build_program() -> bass.Bass:
    nc = bass.Bass(trn_type="TRN2", target_bir_lowering=False, debug=False)

    # x is host-interleaved to [128, nblock, kk, 512] so each 512-col
    # n-block loads in ONE DMA; cos|sin are packed per n-block the same way.
    xTi = nc.dram_tensor("xTi", [128, 4 * 8 * 512], BF16, kind="ExternalInput").ap()
    wqT = nc.dram_tensor("wqT", [128, 8 * 256], BF16, kind="ExternalInput").ap()
    wkT = nc.dram_tensor("wkT", [128, 8 * 64], BF16, kind="ExternalInput").ap()
    wvgT = nc.dram_tensor("wvgT", [128, 8 * 72], BF16, kind="ExternalInput").ap()
    woT = nc.dram_tensor("woT", [256, HM], BF16, kind="ExternalInput").ap()
    csT = nc.dram_tensor("csT", [128, 4 * 1536], BF16, kind="ExternalInput").ap()
    dmask = nc.dram_tensor("dmask", [128, 128], BF16, kind="ExternalInput").ap()
    outp = nc.dram_tensor("outp", [S, HM], FP16, kind="ExternalOutput").ap()

    with tile.TileContext(nc) as tc:
        with (
            tc.tile_pool(name="const", bufs=1) as cpool,
            tc.tile_pool(name="acts", bufs=1) as apool,
        ):
            # ---- persistent tiles; DMAs are emitted inside the n-loop so
            # issue order matches need (reverse n) ----
            wq_all = cpool.tile([128, 8 * 256], BF16, tag="wq")
            wk_all = cpool.tile([128, 8 * 64], BF16, tag="wk")
            wvg_all = cpool.tile([128, 8 * 72], BF16, tag="wvg")
            wvg_sb = [wvg_all[:, i * 72 : (i + 1) * 72] for i in range(8)]
            wk_sb = [wk_all[:, i * 64 : (i + 1) * 64] for i in range(8)]
            wq_sb = [wq_all[:, i * 256 : (i + 1) * 256] for i in range(8)]
            cs_all = cpool.tile([128, 4 * 1536], BF16, tag="cs")
            cos_n = [cs_all[:, n * 1536 : n * 1536 + 512] for n in range(4)]
            sinq_n = [cs_all[:, n * 1536 + 512 : n * 1536 + 1024] for n in range(4)]
            sink_n = [cs_all[:, n * 1536 + 1024 : n * 1536 + 1536] for n in range(4)]
            dmask_sb = cpool.tile([128, 128], BF16, tag="dmask")
            ident_sb = cpool.tile([128, 128], BF16, tag="ident")
            wo_sb = []
            # fp8 (DoubleRow) copies of q/k: [32, 2, S] with the two 32-row
            # feature halves side by side in columns. Heads 0/1 are cast from
            # bf16 by Pool; heads 2/3 are written directly by the rope adds.
            q8_sb = [
                cpool.tile([32, 2 * S], FP8, tag=f"q8_{h}", name=f"q8_{h}")
                for h in range(4)
            ]
            k8_sb = cpool.tile([32, 2 * S], FP8, tag="k8")

            qp01 = apool.tile([128, S], BF16, tag="qp01")
            kT_sb = apool.tile([64, S], BF16, tag="kT")
            v_sb = [
                apool.tile([128, 65], BF16, tag=f"v{s}", name=f"v{s}")
                for s in range(NCH)
            ]
            g_sb = [
                apool.tile([128, 4], FP32, tag=f"g{s}", name=f"g{s}")
                for s in range(NCH)
            ]
            ag_sb = [
                apool.tile([128, 256], BF16, tag=f"ag{s}", name=f"ag{s}")
                for s in range(NCH)
            ]


            # PSUM budget (16KB/partition): st pool is 3 x 4KB slots shared
            # by score tiles and the transpose/out-projection tiles; the
            # projection "mix" pool (2 x 2KB) lives only during the n-loop,
            # and the P@V "acc" pool (2 slots) replaces it afterwards.
            pools = {}
            with (
                tc.tile_pool(name="pt", bufs=56) as ptpool,
                tc.tile_pool(name="rl", bufs=8) as rlpool,
                tc.tile_pool(name="ost", bufs=2) as ostpool,
                tc.tile_pool(name="rope", bufs=4) as rpool,
                tc.tile_pool(name="gtmp", bufs=6) as gpool,
            ):
                def rope(ps, n, parts, qb_on_act):
                    """RoPE a feature-major psum tile ps [parts, 512];
                    returns (m1, rb) bf16 tiles whose sum is the rotated
                    q/k. q blocks are PAIR-INTERLEAVED ([hA-lo|hB-lo|hA-hi|
                    hB-hi]) so the rotate-half is a single +-64 partition
                    shift (2 DVE ops); k keeps the +-32 layout. Rotation
                    signs are folded into the host sin tables; input bases
                    of the two SBUF operands must match, so each sin table
                    is stored half-swapped."""
                    cosv = cos_n[n]
                    qb = rpool.tile([parts, 512], BF16, tag="qb", name="qb")
                    if qb_on_act:
                        nc.scalar.copy(qb[:], ps[:parts, :])
                    else:
                        nc.vector.tensor_copy(qb[:], ps[:parts, :])
                    rb = rpool.tile([parts, 512], BF16, tag="rb", name="rb")
                    m1 = rpool.tile([parts, 512], BF16, tag="m1", name="m1")
                    half = parts // 2
                    sinv = sinq_n[n] if parts == 128 else sink_n[n]
                    nc.vector.tensor_tensor(
                        rb[0:half, :],
                        qb[half : 2 * half, :],
                        sinv[half : 2 * half, :],
                        op=ALU.mult,
                    )
                    nc.vector.tensor_tensor(
                        rb[half : 2 * half, :],
                        qb[0:half, :],
                        sinv[0:half, :],
                        op=ALU.mult,
                    )
                    nc.vector.tensor_tensor(
                        m1[:], qb[:], cosv[:parts, :], op=ALU.mult
                    )
                    return m1, rb

                def emit_k_proj(n, xb, qb_on_act=False):
                    nsl = bass.ts(n, 512)
                    ps = pools["mix"].tile([64, 512], FP32, tag="mix", name="psk")
                    for kk in range(8):
                        nc.tensor.matmul(
                            ps[:],
                            wk_sb[kk][:],
                            xb[:, kk * 512 : (kk + 1) * 512],
                            start=(kk == 0),
                            stop=(kk == 7),
                        )
                    m1, rb = rope(ps, n, 64, qb_on_act=qb_on_act)
                    nc.vector.tensor_tensor(
                        kT_sb[:, nsl], m1[:], rb[:], op=ALU.add
                    )

                def emit_q_proj(m, n, xb, qb_on_act):
                    nsl = bass.ts(n, 512)
                    ps = pools["mix"].tile([128, 512], FP32, tag="mix", name="psq")
                    for kk in range(8):
                        nc.tensor.matmul(
                            ps[:],
                            wq_sb[kk][:, m * 128 : (m + 1) * 128],
                            xb[:, kk * 512 : (kk + 1) * 512],
                            start=(kk == 0),
                            stop=(kk == 7),
                        )
                    m1, rb = rope(ps, n, 128, qb_on_act=qb_on_act)
                    if m == 0:
                        nc.vector.tensor_tensor(
                            qp01[:, nsl], m1[:], rb[:], op=ALU.add
                        )
                    else:
                        # pair-interleaved rows: head hh's halves are at
                        # partitions [hh*32 : hh*32+32] and [64+hh*32 : ...]
                        for hh in range(2):
                            for half in range(2):
                                p0 = half * 64 + hh * 32
                                nc.vector.tensor_tensor(
                                    q8_sb[2 + hh][
                                        :, half * S + n * 512 : half * S + (n + 1) * 512
                                    ],
                                    m1[p0 : p0 + 32, :],
                                    rb[p0 : p0 + 32, :],
                                    op=ALU.add,
                                )

                def emit_vg(n, xb):
                    # v + gates, token-major; the 4 sq-chunks of this n-block
                    # share one psum tile. Gate tanh reads a DVE-staged SBUF
                    # copy so the psum slot frees without waiting on ACT.
                    ps = pools["mix"].tile([128, 288], FP32, tag="mix", name="psvg")
                    for sub in range(4):
                        s = 4 * n + sub
                        for kk in range(8):
                            nc.tensor.matmul(
                                ps[:, sub * 72 : (sub + 1) * 72],
                                xb[:, kk * 512 + sub * 128 : kk * 512 + (sub + 1) * 128],
                                wvg_sb[kk][:],
                                start=(sub == 0 and kk == 0),
                                stop=(sub == 3 and kk == 7),
                            )
                    gsb = gpool.tile([128, 32], FP32, tag="gsb", name="gsb")
                    psv = ps[:].rearrange("p (four c) -> p four c", four=4)
                    nc.vector.tensor_copy(
                        gsb[:].rearrange("p (four c) -> p four c", four=4),
                        psv[:, :, 64:72],
                    )
                    tg = gpool.tile([128, 32], FP32, tag="tg", name="tg")
                    nc.scalar.activation(tg[:], gsb[:], AF.Tanh, scale=0.5)
                    for sub in range(4):
                        s = 4 * n + sub
                        o = sub * 72
                        nc.vector.tensor_copy(v_sb[s][:, 0:64], ps[:, o : o + 64])
                        nc.vector.memset(v_sb[s][:, 64:65], 1.0)
                        # gate: G = 1 + 0.5*(tanh(a/2) + tanh(b/2))
                        gs = gpool.tile([128, 4], FP32, tag="gs", name="gs")
                        nc.gpsimd.tensor_tensor(
                            gs[:], tg[:, sub * 8 : sub * 8 + 4],
                            tg[:, sub * 8 + 4 : sub * 8 + 8], op=ALU.add
                        )
                        nc.gpsimd.tensor_scalar(
                            g_sb[s][:], gs[:], 0.5, 1.0,
                            op0=ALU.mult, op1=ALU.add,
                        )

                def cast_q8(q8t, src, n, r0):
                    # src rows [r0:r0+32] = lo half, [r0+64:r0+96] = hi half
                    nsl = bass.ts(n, 512)
                    nc.gpsimd.tensor_copy(
                        q8t[:, n * 512 : (n + 1) * 512], src[r0 : r0 + 32, nsl]
                    )
                    nc.gpsimd.tensor_copy(
                        q8t[:, S + n * 512 : S + (n + 1) * 512],
                        src[r0 + 64 : r0 + 96, nsl],
                    )

                def cast_k8(q8t, src, n):
                    nsl = bass.ts(n, 512)
                    nc.gpsimd.tensor_copy(
                        q8t[:, n * 512 : (n + 1) * 512], src[0:32, nsl]
                    )
                    nc.gpsimd.tensor_copy(
                        q8t[:, S + n * 512 : S + (n + 1) * 512], src[32:64, nsl]
                    )

                def cast_q8_piece(q8t, src, p):
                    # [32,1024] pieces for the late heads (emitted interleaved
                    # so Pool's queue never blocks the dmask multiplies)
                    psl = bass.ts(p, 1024)
                    half = 0 if p < 2 else 1
                    pp = p % 2
                    nc.gpsimd.tensor_copy(
                        q8t[:, half * S + pp * 1024 : half * S + (pp + 1) * 1024],
                        src[half * 32 : half * 32 + 32, pp * 1024 : (pp + 1) * 1024],
                    )

                def emit_scores_fp8(h, c, q8t):
                    width = S - c * 128
                    k8v = k8_sb[:].rearrange("p (two n) -> p two n", two=2)
                    q8v = q8t[:].rearrange("p (two n) -> p two n", two=2)
                    pts_ = []
                    for t0 in range(0, width, 1024):
                        cols = min(1024, width - t0)
                        st = pools["st"].tile([128, 1024], FP32, tag="st", name="st")
                        pt = ptpool.tile([128, 1024], BF16, tag="pt", name="pt")
                        pts_.append(pt)
                        for n0 in range(0, cols, 256):
                            nn = min(256, cols - n0)
                            lo = c * 128 + t0 + n0
                            nc.tensor.matmul(
                                st[:, n0 : n0 + nn],
                                k8v[:, :, c * 128 : (c + 1) * 128],
                                q8v[:, :, lo : lo + nn],
                                start=True,
                                stop=True,
                                perf_mode=PM.DoubleRow,
                            )
                        nc.scalar.activation(
                            pt[:, 0:cols], st[:, 0:cols], AF.Exp, scale=0.125
                        )
                        if t0 == 0:
                            nc.gpsimd.tensor_tensor(
                                pt[:, 0:128], pt[:, 0:128], dmask_sb[:],
                                op=ALU.mult,
                            )
                    return pts_

                def emit_pv(h, s, pts_by_c, accv, first):
                    """P@V over kv chunks for one sq chunk into accv
                    [128, 65] (col 64 = softmax denominator)."""
                    for c in range(s + 1):
                        off = (s - c) * 128
                        nc.tensor.matmul(
                            accv[:, 0:65],
                            pts_by_c[c][off // 1024][:, off % 1024 : off % 1024 + 128],
                            v_sb[c][:],
                            start=(c == 0),
                            stop=(c == s),
                        )

                def emit_sq_pair(items):
                    """One or two (h, s) P@V+epilogue ops sharing a single
                    acc psum tile; head-3 entries carry the transpose +
                    output projection + DMA (fp16 partials)."""
                    acc = pools["acc"].tile([128, 130], FP32, tag="acc", name="acc")
                    for idx, (h, s, ptl) in enumerate(items):
                        emit_pv(h, s, ptl, acc[:, idx * 65 : idx * 65 + 65], idx == 0)
                    rl = rlpool.tile([128, 2], FP32, tag="rl", name="rl")
                    accv = acc[:].rearrange("p (two c) -> p two c", two=2)
                    n_it = len(items)
                    nc.vector.reciprocal(
                        rl[:, 0:n_it],
                        accv[:, 0:n_it, 64:65],
                    )
                    for idx, (h, s, ptl) in enumerate(items):
                        nc.vector.tensor_scalar(
                            ag_sb[s][:, h * 64 : (h + 1) * 64],
                            acc[:, idx * 65 : idx * 65 + 64],
                            rl[:, idx : idx + 1],
                            g_sb[s][:, h : h + 1],
                            op0=ALU.mult,
                            op1=ALU.mult,
                        )
                    for h, s, ptl in items:
                        if h != 3:
                            continue
                        agT = [
                            gpool.tile([128, 128], BF16, tag="agT", name="agT")
                            for _ in range(2)
                        ]
                        for j in range(2):
                            tp = pools["st"].tile([128, 128], BF16, tag="st", name="tp")
                            nc.tensor.transpose(
                                tp[:],
                                ag_sb[s][:, j * 128 : (j + 1) * 128],
                                ident_sb[:],
                            )
                            nc.vector.tensor_copy(agT[j][:], tp[:])
                        ost = ostpool.tile([128, HM], FP16, tag="ost", name="ost")
                        for nn in range(2):
                            po = pools["st"].tile([128, 512], FP32, tag="st", name="po")
                            for j in range(2):
                                nc.tensor.matmul(
                                    po[:],
                                    agT[j][:],
                                    wo_sb[j][:, nn * 512 : (nn + 1) * 512],
                                    start=(j == 0),
                                    stop=(j == 1),
                                )
                            if s >= 14:
                                nc.scalar.copy(
                                    ost[:, nn * 512 : (nn + 1) * 512], po[:]
                                )
                            else:
                                nc.vector.tensor_copy(
                                    ost[:, nn * 512 : (nn + 1) * 512], po[:]
                                )
                            if s >= 14:
                                nc.sync.dma_start(
                                    outp[s * 128 : (s + 1) * 128,
                                         nn * 512 : (nn + 1) * 512],
                                    ost[:, nn * 512 : (nn + 1) * 512],
                                )
                        if s < 14:
                            nc.sync.dma_start(
                                outp[s * 128 : (s + 1) * 128, :], ost[:]
                            )

                # ---- emission ----
                # Startup: stream x/cos/sin in REVERSE n-block order; after
                # block n arrives, project k/q0/q1 for it and immediately
                # emit head-0 (and head-1 for n>=1) scores for the c-chunks
                # that only need blocks >= n (ascending c within each batch
                # so exp(h,low c) lands first). This keeps ACT fed from ~5us.
                def ckpt(label):
                    PHASELOG.append((label, nc.next_id()))

                nc.sync.dma_start(wk_all[:], wkT[:, :])
                pts = {h: [None] * NCH for h in range(4)}
                mix_cm = tc.tile_pool(name="mix", bufs=3, space="PSUM")
                pools["mix"] = mix_cm.__enter__()
                stA_cm = tc.tile_pool(name="stA", bufs=2, space="PSUM")
                pools["st"] = stA_cm.__enter__()
                xpool_cm = tc.tile_pool(name="xp", bufs=2)
                xpool = xpool_cm.__enter__()
                for n in (3, 2, 1, 0):
                    ckpt(f"nloop{n}")
                    nc.sync.dma_start(
                        cs_all[:, n * 1536 : (n + 1) * 1536],
                        csT[:, n * 1536 : (n + 1) * 1536],
                    )
                    xb = xpool.tile([128, 8 * 512], BF16, tag="xb", name=f"xb{n}")
                    nc.sync.dma_start(
                        xb[:, 0 : 4 * 512], xTi[:, n * 4096 : n * 4096 + 2048]
                    )
                    nc.sync.dma_start(
                        xb[:, 4 * 512 : 8 * 512],
                        xTi[:, n * 4096 + 2048 : (n + 1) * 4096],
                    )
                    if n == 3:
                        nc.sync.dma_start(wq_all[:], wqT[:, :])
                        nc.sync.dma_start(wvg_all[:], wvgT[:, :])
                        nc.sync.dma_start(dmask_sb[:], dmask[:, :])
                        make_identity(nc, ident_sb[:])
                    emit_k_proj(n, xb, qb_on_act=(n == 3))
                    cast_k8(k8_sb, kT_sb, n)
                    emit_q_proj(0, n, xb, qb_on_act=(n == 3))
                    cast_q8(q8_sb[0], qp01, n, 0)
                    cast_q8(q8_sb[1], qp01, n, 32)
                    pts[0][4 * n] = emit_scores_fp8(0, 4 * n, q8_sb[0])
                    pts[0][4 * n + 1] = emit_scores_fp8(0, 4 * n + 1, q8_sb[0])
                    emit_q_proj(1, n, xb, qb_on_act=(n == 3))
                    pts[0][4 * n + 2] = emit_scores_fp8(0, 4 * n + 2, q8_sb[0])
                    pts[0][4 * n + 3] = emit_scores_fp8(0, 4 * n + 3, q8_sb[0])
                    emit_vg(n, xb)
                    if n >= 1:
                        for c in range(4 * n, 4 * n + 4):
                            pts[1][c] = emit_scores_fp8(1, c, q8_sb[1])
                    if n >= 2:
                        for c in range(4 * n, 4 * n + 4):
                            pts[2][c] = emit_scores_fp8(2, c, q8_sb[2])
                xpool_cm.__exit__(None, None, None)
                late_cm = tc.tile_pool(name="late", bufs=1)
                late = late_cm.__enter__()
                wo_sb.append(late.tile([128, HM], BF16, tag="wo0", name="wo0"))
                wo_sb.append(late.tile([128, HM], BF16, tag="wo1", name="wo1"))
                for j in range(2):
                    nc.sync.dma_start(wo_sb[j][:], woT[j * 128 : (j + 1) * 128, :])

                stA_cm.__exit__(None, None, None)
                mix_cm.__exit__(None, None, None)
                stB_cm = tc.tile_pool(name="stB", bufs=3, space="PSUM")
                pools["st"] = stB_cm.__enter__()
                acc_cm = tc.tile_pool(name="acc", bufs=2, space="PSUM")
                pools["acc"] = acc_cm.__enter__()

                # Phase A: finish h1's wide chunks, then drain h0+h1 P@V
                # (their exps are already queued on ACT).
                ckpt("A-sc1")
                for c in range(4):
                    pts[1][c] = emit_scores_fp8(1, c, q8_sb[1])
                for c in range(NCH):
                    ckpt(f"A-sq{c}")
                    emit_sq_pair([(0, c, pts[0])])
                    emit_sq_pair([(1, c, pts[1])])

                # Phase B: h2/h3 scores stream at full rate (exp supply for
                # ACT), both sq streams 2 chunks behind; h3's sq carries the
                # output projection so that work spreads across the phase.
                for c in range(18):
                    if c <= 7:
                        ckpt(f"B{c}")
                        pts[2][c] = emit_scores_fp8(2, c, q8_sb[2])
                    if c <= 15:
                        pts[3][c] = emit_scores_fp8(3, c, q8_sb[3])
                    if 2 <= c <= 17:
                        emit_sq_pair([(2, c - 2, pts[2])])
                        emit_sq_pair([(3, c - 2, pts[3])])
                acc_cm.__exit__(None, None, None)
                stB_cm.__exit__(None, None, None)
                late_cm.__exit__(None, None, None)

    _split_multi_waits(nc)
    return nc


_NC = None


def _get_nc() -> bass.Bass:
    global _NC
    if _NC is None:
        _NC = _build_program()
    return _NC


def _shard_inputs(
    hidden_states, Wq, Wk, Wv, Wo, Wkc, Wg_slc, Wg_swa
) -> list[dict[str, np.ndarray]]:
    bf16 = ml_dtypes.bfloat16
    f32 = np.float32

    # RoPE tables (bf16, feature-major, duplicated across two 64-row head
    # blocks). The rotation's half-swap is fused into the device multiply's
    # input AP, so the sin table here is stored half-SWAPPED and signed:
    # sinP[d] = +sin for d<32 (pairs with src d+32... device reads
    # sin[src-partition]), -sin for d>=32. Since sin/cos rows repeat with
    # period 32, the swap is a sign flip on the upper half.
    inv = 1.0 / (THETA ** (np.arange(0, D, 2, dtype=np.float64) / D))
    freqs = np.arange(S, dtype=np.float64)[:, None] * inv  # [S, 32]
    cos32 = np.cos(freqs).T  # [32, S]
    sin32 = np.sin(freqs).T
    # cos repeats every 32 rows (valid for both the pair-interleaved q
    # layout and the k layout). The sin tables are stored half-SWAPPED and
    # signed so the device's shifted multiply reads the right value at the
    # SOURCE partition: sinQ pairs rows p <-> p+64, sinK pairs p <-> p+32.
    cos2 = np.concatenate([cos32] * 4, axis=0).astype(bf16)  # [128, S]
    sinQ = np.concatenate([sin32, sin32, -sin32, -sin32], axis=0).astype(bf16)
    sinK = np.concatenate([sin32, -sin32, sin32, -sin32], axis=0).astype(bf16)
    csT = np.concatenate(
        [
            np.concatenate(
                [
                    cos2[:, n * 512 : (n + 1) * 512],
                    sinQ[:, n * 512 : (n + 1) * 512],
                    sinK[:, n * 512 : (n + 1) * 512],
                ],
                axis=1,
            )
            for n in range(4)
        ],
        axis=1,
    )
    csT = np.ascontiguousarray(csT)

    # pt[kv_i, sq_j] is valid iff kv <= sq, i.e. i <= j: upper triangular
    dmask = np.triu(np.ones((128, 128), dtype=f32)).astype(bf16)

    def interleave(w):
        """[1024, width] -> [128, 8*width] with hm-chunk-major columns so
        the whole weight loads in one contiguous DMA."""
        width = w.shape[1]
        return np.ascontiguousarray(
            w.reshape(8, 128, width).transpose(1, 0, 2).reshape(128, 8 * width)
        )

    in_maps = []
    for core in range(NCORES):
        b, g = divmod(core, 4)
        # xTi[p, n, kk, j] = x.T[kk*128+p, n*512+j]
        xTc = (
            np.ascontiguousarray(hidden_states[b].T)
            .astype(bf16)
            .reshape(8, 128, 4, 512)
            .transpose(1, 2, 0, 3)
            .reshape(128, 4 * 8 * 512)
        )
        xTc = np.ascontiguousarray(xTc)
        # pair-interleave each m-block's output features: rows become
        # [hA d0:32 | hB d0:32 | hA d32:64 | hB d32:64] for heads (2m, 2m+1)
        wq_g = Wq[g * 256 : (g + 1) * 256, :].reshape(2, 2, 2, 32, HM)
        # [m, head, half, 32, HM] -> [m, half, head, 32, HM]
        wq_g = np.ascontiguousarray(wq_g.transpose(0, 2, 1, 3, 4)).reshape(256, HM)
        wqTc = interleave(np.ascontiguousarray(wq_g.T).astype(bf16))
        wkTc = interleave(
            np.ascontiguousarray(Wk[g * 64 : (g + 1) * 64, :].T).astype(bf16)
        )
        wvg = np.concatenate(
            [
                Wv[g * 64 : (g + 1) * 64, :].T,
                Wg_slc[g * 4 : (g + 1) * 4, :].T,
                Wg_swa[g * 4 : (g + 1) * 4, :].T,
            ],
            axis=1,
        )  # [1024, 72]
        wvgc = interleave(np.ascontiguousarray(wvg).astype(bf16))
        woTc = np.ascontiguousarray(Wo[:, g * 256 : (g + 1) * 256].T).astype(bf16)
        in_maps.append(
            {
                "xTi": xTc,
                "wqT": wqTc,
                "wkT": wkTc,
                "wvgT": wvgc,
                "woT": woTc,
                "csT": csT,
                "dmask": dmask,
            }
        )
    return in_maps


def run(inputs: dict, trace: bool = False):
    """Run the SPMD kernel; returns (output [B,S,HM] f32, BassKernelResults)."""
    nc = _get_nc()
    in_maps = _shard_inputs(**inputs)
    res = run_bass_kernel_spmd(
        nc, in_maps, core_ids=list(range(NCORES)), trace=trace
    )
    out = np.zeros((B, S, HM), np.float32)
    for core in range(NCORES):
        b = core // 4
        out[b] += res.results[core]["outp"].astype(np.float32)
    return out, res


def kernel(**inputs) -> np.ndarray:
    out, _ = run(inputs)
    return out


# revision 56
# speedup vs baseline: 1.0287x; 1.0287x over previous
"""NativeSparseAttention (fallback = full causal SDPA) Trainium2 kernel.

Sharding: 8 cores = 2 (batch) x 4 (kv head groups). Core (b, g) computes
q heads 4g..4g+3, kv head g, batch b, and a row-parallel partial of the
output projection; partials are summed on the host (the "all-reduce").

Schedule (v2): x/cos/sin stream in REVERSE column-block order (n=3..0) so
projections + head-0 scores start ~3us in; heads 1-3 stream scores 2
chunks ahead of P@V; head-3 sq carries the output projection; output is
fp16 partials. exp bottlenecks ACT (~58us floor), so everything else is
spread across PE/DVE/Pool.

Layouts on device (per core):
  xT    [1024, 2048] bf16   hidden_states[b].T (4 col-block DMAs per tile)
  qT    [256, 2048]  bf16   feature-major q (RoPE applied)
  kT    [64, 2048]   bf16   feature-major k (RoPE applied)
  v     [2048, 65]   bf16   token-major v with ones column (softmax denom)
  pT    [kv, sq]            scores transposed; exp(st/8) on ACT; diag mask
  attn  [sq, 65]     f32    PSUM accumulated over kv chunks; col 64 = denom
  ag    [2048, 256]  bf16   gated/normalized attn, token-major
  agT   [256, 2048]  bf16   PE-transposed for output projection
  outp  [2048, 1024] fp16   partial output
"""

import numpy as np
import ml_dtypes

import concourse.bass as bass
import concourse.mybir as mybir
import concourse.tile as tile
from concourse.bass_utils import run_bass_kernel_spmd
from concourse.masks import make_identity

FP32 = mybir.dt.float32
FP16 = mybir.dt.float16
BF16 = mybir.dt.bfloat16
FP8 = mybir.dt.float8e4
PM = mybir.MatmulPerfMode
AF = mybir.ActivationFunctionType
ALU = mybir.AluOpType


def _patch_tail_drain():
    """This container's walrus build allows only ONE semaphore wait per CTRL
    (Drain/NoOp) instruction, but Tile's kernel-tail drain attaches one wait
    per active queue/engine. Split the waits across preceding single-wait
    NOPs on the same engine (SP executes them in order, so semantics are
    unchanged)."""
    from bass_rust import ScopedClock

    if getattr(tile.TileContext, "_tail_drain_patched", False):
        return

    def _drain_and_barrier(self, tick_clock, wait_clock):
        nc = self.nc
        probe = nc.sync.nop(nofuse=True)
        wait_clock.add_sem_waits(
            probe.ins, ScopedClock({None: tick_clock.global_clock})
        )
        si = probe.ins.sync_info
        waits = list(si.on_wait) if si is not None else []
        if len(waits) > 1:
            si.on_wait = waits[:1]
            for w in waits[1:]:
                n2 = nc.sync.nop(nofuse=True)
                n2.ins.sync_info = mybir.SyncInfo(on_wait=[w], on_update=[])
        nc.sync.drain()
        nc.all_engine_barrier()
        popped = nc._tile_sem_poison_stack.pop()
        assert popped is self._sem_poison
        nc.clear_and_free_semaphores(list(self.sems.allocated().values()))
        nc.all_engine_barrier()

    tile.TileContext._drain_and_barrier = _drain_and_barrier
    tile.TileContext._tail_drain_patched = True


_patch_tail_drain()

PHASELOG = []

B = 2
S = 2048
HM = 1024
NH = 16
NKV = 4
D = 64
THETA = 10000.0
NCORES = 8

NCH = S // 128  # 16 sequence chunks of 128


def _split_multi_waits(nc: bass.Bass):
    """Walrus here allows a single semaphore wait per instruction; hoist
    extra waits onto same-engine NOPs placed immediately before (same
    sequencer, in-order => identical semantics)."""
    for f in nc.m.functions:
        for b in f.blocks:
            new = []
            changed = False
            for ins in b.instructions:
                si = ins.sync_info
                waits = list(si.on_wait) if si is not None else []
                if len(waits) > 1:
                    changed = True
                    for i, w in enumerate(waits[:-1]):
                        nop = mybir.InstNoOp(
                            name=f"{ins.name}-sw{i}",
                            sync_info=mybir.SyncInfo(on_wait=[w], on_update=[]),
                            bass_nofuse=True,
                            engine=ins.engine,
                        )
                        nc.register_instruction(nop, overwrite=True)
                        new.append(nop)
                    si.on_wait = waits[-1:]
                new.append(ins)
            if changed:
                b.instructions = new



def _build_program() -> bass.Bass:
    nc = bass.Bass(trn_type="TRN2", target_bir_lowering=False, debug=False)

    # x is host-interleaved to [128, nblock, kk, 512] so each 512-col
    # n-block loads in ONE DMA; cos|sin are packed per n-block the same way.
    xTi = nc.dram_tensor("xTi", [128, 4 * 8 * 512], BF16, kind="ExternalInput").ap()
    wqT = nc.dram_tensor("wqT", [128, 8 * 256], BF16, kind="ExternalInput").ap()
    wkT = nc.dram_tensor("wkT", [128, 8 * 64], BF16, kind="ExternalInput").ap()
    wvgT = nc.dram_tensor("wvgT", [128, 8 * 72], BF16, kind="ExternalInput").ap()
    woT = nc.dram_tensor("woT", [256, HM], BF16, kind="ExternalInput").ap()
    csT = nc.dram_tensor("csT", [128, 4 * 1536], BF16, kind="ExternalInput").ap()
    dmask = nc.dram_tensor("dmask", [128, 128], BF16, kind="ExternalInput").ap()
    outp = nc.dram_tensor("outp", [S, HM], FP16, kind="ExternalOutput").ap()

    with tile.TileContext(nc) as tc:
        with (
            tc.tile_pool(name="const", bufs=1) as cpool,
            tc.tile_pool(name="acts", bufs=1) as apool,
        ):
            # ---- persistent tiles; DMAs are emitted inside the n-loop so
            # issue order matches need (reverse n) ----
            wq_all = cpool.tile([128, 8 * 256], BF16, tag="wq")
            wk_all = cpool.tile([128, 8 * 64], BF16, tag="wk")
            wvg_all = cpool.tile([128, 8 * 72], BF16, tag="wvg")
            wvg_sb = [wvg_all[:, i * 72 : (i + 1) * 72] for i in range(8)]
            wk_sb = [wk_all[:, i * 64 : (i + 1) * 64] for i in range(8)]
            wq_sb = [wq_all[:, i * 256 : (i + 1) * 256] for i in range(8)]
            cs_all = cpool.tile([128, 4 * 1536], BF16, tag="cs")
            cos_n = [cs_all[:, n * 1536 : n * 1536 + 512] for n in range(4)]
            sinq_n = [cs_all[:, n * 1536 + 512 : n * 1536 + 1024] for n in range(4)]
            sink_n = [cs_all[:, n * 1536 + 1024 : n * 1536 + 1536] for n in range(4)]
            dmask_sb = cpool.tile([128, 128], BF16, tag="dmask")
            ident_sb = cpool.tile([128, 128], BF16, tag="ident")
            wo_sb = []
            # fp8 (DoubleRow) copies of q/k: [32, 2, S] with the two 32-row
            # feature halves side by side in columns. Heads 0/1 are cast from
            # bf16 by Pool; heads 2/3 are written directly by the rope adds.
            q8_sb = [
                cpool.tile([32, 2 * S], FP8, tag=f"q8_{h}", name=f"q8_{h}")
                for h in range(4)
            ]
            k8_sb = cpool.tile([32, 2 * S], FP8, tag="k8")

            qp01 = apool.tile([128, S], BF16, tag="qp01")
            kT_sb = apool.tile([64, S], BF16, tag="kT")
            v_sb = [
                apool.tile([128, 65], BF16, tag=f"v{s}", name=f"v{s}")
                for s in range(NCH)
            ]
            g_sb = [
                apool.tile([128, 4], FP32, tag=f"g{s}", name=f"g{s}")
                for s in range(NCH)
            ]
            gsb_all = apool.tile([128, 128], FP32, tag="gsball")
            tg_all = apool.tile([128, 128], FP32, tag="tgall")
            ag_sb = [
                apool.tile([128, 256], BF16, tag=f"ag{s}", name=f"ag{s}")
                for s in range(NCH)
            ]


            # PSUM budget (16KB/partition): st pool is 3 x 4KB slots shared
            # by score tiles and the transpose/out-projection tiles; the
            # projection "mix" pool (2 x 2KB) lives only during the n-loop,
            # and the P@V "acc" pool (2 slots) replaces it afterwards.
            pools = {}
            with (
                tc.tile_pool(name="st", bufs=3, space="PSUM") as stpool,
                tc.tile_pool(name="pt", bufs=56) as ptpool,
                tc.tile_pool(name="rl", bufs=8) as rlpool,
                tc.tile_pool(name="ost", bufs=2) as ostpool,
                tc.tile_pool(name="rope", bufs=4) as rpool,
                tc.tile_pool(name="gtmp", bufs=6) as gpool,
            ):
                def rope(ps, n, parts, qb_on_act):
                    """RoPE a feature-major psum tile ps [parts, 512];
                    returns (m1, rb) bf16 tiles whose sum is the rotated
                    q/k. q blocks are PAIR-INTERLEAVED ([hA-lo|hB-lo|hA-hi|
                    hB-hi]) so the rotate-half is a single +-64 partition
                    shift (2 DVE ops); k keeps the +-32 layout. Rotation
                    signs are folded into the host sin tables; input bases
                    of the two SBUF operands must match, so each sin table
                    is stored half-swapped."""
                    cosv = cos_n[n]
                    qb = rpool.tile([parts, 512], BF16, tag="qb", name="qb")
                    if qb_on_act:
                        nc.scalar.copy(qb[:], ps[:parts, :])
                    else:
                        nc.vector.tensor_copy(qb[:], ps[:parts, :])
                    rb = rpool.tile([parts, 512], BF16, tag="rb", name="rb")
                    m1 = rpool.tile([parts, 512], BF16, tag="m1", name="m1")
                    half = parts // 2
                    sinv = sinq_n[n] if parts == 128 else sink_n[n]
                    nc.vector.tensor_tensor(
                        rb[0:half, :],
                        qb[half : 2 * half, :],
                        sinv[half : 2 * half, :],
                        op=ALU.mult,
                    )
                    nc.vector.tensor_tensor(
                        rb[half : 2 * half, :],
                        qb[0:half, :],
                        sinv[0:half, :],
                        op=ALU.mult,
                    )
                    nc.vector.tensor_tensor(
                        m1[:], qb[:], cosv[:parts, :], op=ALU.mult
                    )
                    return m1, rb

                def emit_k_proj(n, xb, qb_on_act=False):
                    nsl = bass.ts(n, 512)
                    ps = pools["mix"].tile([64, 512], FP32, tag="mix", name="psk")
                    for kk in range(8):
                        nc.tensor.matmul(
                            ps[:],
                            wk_sb[kk][:],
                            xb[:, kk * 512 : (kk + 1) * 512],
                            start=(kk == 0),
                            stop=(kk == 7),
                        )
                    m1, rb = rope(ps, n, 64, qb_on_act=qb_on_act)
                    nc.vector.tensor_tensor(
                        kT_sb[:, nsl], m1[:], rb[:], op=ALU.add
                    )

                def emit_q_proj(m, n, xb, qb_on_act):
                    nsl = bass.ts(n, 512)
                    ps = pools["mix"].tile([128, 512], FP32, tag="mix", name="psq")
                    for kk in range(8):
                        nc.tensor.matmul(
                            ps[:],
                            wq_sb[kk][:, m * 128 : (m + 1) * 128],
                            xb[:, kk * 512 : (kk + 1) * 512],
                            start=(kk == 0),
                            stop=(kk == 7),
                        )
                    m1, rb = rope(ps, n, 128, qb_on_act=qb_on_act)
                    if m == 0:
                        nc.vector.tensor_tensor(
                            qp01[:, nsl], m1[:], rb[:], op=ALU.add
                        )
                    else:
                        # pair-interleaved rows: head hh's halves are at
                        # partitions [hh*32 : hh*32+32] and [64+hh*32 : ...]
                        for hh in range(2):
                            for half in range(2):
                                p0 = half * 64 + hh * 32
                                nc.vector.tensor_tensor(
                                    q8_sb[2 + hh][
                                        :, half * S + n * 512 : half * S + (n + 1) * 512
                                    ],
                                    m1[p0 : p0 + 32, :],
                                    rb[p0 : p0 + 32, :],
                                    op=ALU.add,
                                )

                def emit_vg(n, xb):
                    # v + gates, token-major; the 4 sq-chunks of this n-block
                    # share one psum tile. Gate tanh reads a DVE-staged SBUF
                    # copy so the psum slot frees without waiting on ACT.
                    ps = pools["mix"].tile([128, 288], FP32, tag="mix", name="psvg")
                    for sub in range(4):
                        s = 4 * n + sub
                        for kk in range(8):
                            nc.tensor.matmul(
                                ps[:, sub * 72 : (sub + 1) * 72],
                                xb[:, kk * 512 + sub * 128 : kk * 512 + (sub + 1) * 128],
                                wvg_sb[kk][:],
                                start=(sub == 0 and kk == 0),
                                stop=(sub == 3 and kk == 7),
                            )
                    psv = ps[:].rearrange("p (four c) -> p four c", four=4)
                    nc.vector.tensor_copy(
                        gsb_all[:, n * 32 : (n + 1) * 32].rearrange(
                            "p (four c) -> p four c", four=4
                        ),
                        psv[:, :, 64:72],
                    )
                    for sub in range(4):
                        s = 4 * n + sub
                        o = sub * 72
                        nc.vector.tensor_copy(v_sb[s][:, 0:64], ps[:, o : o + 64])
                        nc.vector.memset(v_sb[s][:, 64:65], 1.0)

                def cast_q8(q8t, src, n, r0):
                    # src rows [r0:r0+32] = lo half, [r0+64:r0+96] = hi half
                    nsl = bass.ts(n, 512)
                    nc.gpsimd.tensor_copy(
                        q8t[:, n * 512 : (n + 1) * 512], src[r0 : r0 + 32, nsl]
                    )
                    nc.gpsimd.tensor_copy(
                        q8t[:, S + n * 512 : S + (n + 1) * 512],
                        src[r0 + 64 : r0 + 96, nsl],
                    )

                def cast_k8(q8t, src, n):
                    nsl = bass.ts(n, 512)
                    nc.gpsimd.tensor_copy(
                        q8t[:, n * 512 : (n + 1) * 512], src[0:32, nsl]
                    )
                    nc.gpsimd.tensor_copy(
                        q8t[:, S + n * 512 : S + (n + 1) * 512], src[32:64, nsl]
                    )

                def cast_q8_piece(q8t, src, p):
                    # [32,1024] pieces for the late heads (emitted interleaved
                    # so Pool's queue never blocks the dmask multiplies)
                    psl = bass.ts(p, 1024)
                    half = 0 if p < 2 else 1
                    pp = p % 2
                    nc.gpsimd.tensor_copy(
                        q8t[:, half * S + pp * 1024 : half * S + (pp + 1) * 1024],
                        src[half * 32 : half * 32 + 32, pp * 1024 : (pp + 1) * 1024],
                    )

                def emit_scores_fp8(h, c, q8t):
                    width = S - c * 128
                    k8v = k8_sb[:].rearrange("p (two n) -> p two n", two=2)
                    q8v = q8t[:].rearrange("p (two n) -> p two n", two=2)
                    pts_ = []
                    for t0 in range(0, width, 1024):
                        cols = min(1024, width - t0)
                        st = stpool.tile([128, 1024], FP32, tag="st", name="st")
                        pt = ptpool.tile([128, 1024], BF16, tag="pt", name="pt")
                        pts_.append(pt)
                        for n0 in range(0, cols, 256):
                            nn = min(256, cols - n0)
                            lo = c * 128 + t0 + n0
                            nc.tensor.matmul(
                                st[:, n0 : n0 + nn],
                                k8v[:, :, c * 128 : (c + 1) * 128],
                                q8v[:, :, lo : lo + nn],
                                start=True,
                                stop=True,
                                perf_mode=PM.DoubleRow,
                            )
                        nc.scalar.activation(
                            pt[:, 0:cols], st[:, 0:cols], AF.Exp, scale=0.125
                        )
                        if t0 == 0:
                            nc.gpsimd.tensor_tensor(
                                pt[:, 0:128], pt[:, 0:128], dmask_sb[:],
                                op=ALU.mult,
                            )
                    return pts_

                def emit_pv(h, s, pts_by_c, accv, first):
                    """P@V over kv chunks for one sq chunk into accv
                    [128, 65] (col 64 = softmax denominator)."""
                    for c in range(s + 1):
                        off = (s - c) * 128
                        nc.tensor.matmul(
                            accv[:, 0:65],
                            pts_by_c[c][off // 1024][:, off % 1024 : off % 1024 + 128],
                            v_sb[c][:],
                            start=(c == 0),
                            stop=(c == s),
                        )

                def emit_sq_pair(items):
                    """One or two (h, s) P@V+epilogue ops sharing a single
                    acc psum tile; head-3 entries carry the transpose +
                    output projection + DMA (fp16 partials)."""
                    acc = pools["acc"].tile([128, 130], FP32, tag="acc", name="acc")
                    for idx, (h, s, ptl) in enumerate(items):
                        emit_pv(h, s, ptl, acc[:, idx * 65 : idx * 65 + 65], idx == 0)
                    rl = rlpool.tile([128, 2], FP32, tag="rl", name="rl")
                    accv = acc[:].rearrange("p (two c) -> p two c", two=2)
                    n_it = len(items)
                    nc.vector.reciprocal(
                        rl[:, 0:n_it],
                        accv[:, 0:n_it, 64:65],
                    )
                    for idx, (h, s, ptl) in enumerate(items):
                        nc.vector.tensor_scalar(
                            ag_sb[s][:, h * 64 : (h + 1) * 64],
                            acc[:, idx * 65 : idx * 65 + 64],
                            rl[:, idx : idx + 1],
                            g_sb[s][:, h : h + 1],
                            op0=ALU.mult,
                            op1=ALU.mult,
                        )
                    for h, s, ptl in items:
                        if h != 3:
                            continue
                        agT = [
                            gpool.tile([128, 128], BF16, tag="agT", name="agT")
                            for _ in range(2)
                        ]
                        for j in range(2):
                            tp = stpool.tile([128, 128], BF16, tag="st", name="tp")
                            nc.tensor.transpose(
                                tp[:],
                                ag_sb[s][:, j * 128 : (j + 1) * 128],
                                ident_sb[:],
                            )
                            nc.vector.tensor_copy(agT[j][:], tp[:])
                        ost = ostpool.tile([128, HM], FP16, tag="ost", name="ost")
                        for nn in range(2):
                            po = stpool.tile([128, 512], FP32, tag="st", name="po")
                            for j in range(2):
                                nc.tensor.matmul(
                                    po[:],
                                    agT[j][:],
                                    wo_sb[j][:, nn * 512 : (nn + 1) * 512],
                                    start=(j == 0),
                                    stop=(j == 1),
                                )
                            if s >= 14:
                                nc.scalar.copy(
                                    ost[:, nn * 512 : (nn + 1) * 512], po[:]
                                )
                            else:
                                nc.vector.tensor_copy(
                                    ost[:, nn * 512 : (nn + 1) * 512], po[:]
                                )
                            if s >= 14:
                                nc.sync.dma_start(
                                    outp[s * 128 : (s + 1) * 128,
                                         nn * 512 : (nn + 1) * 512],
                                    ost[:, nn * 512 : (nn + 1) * 512],
                                )
                        if s < 14:
                            nc.sync.dma_start(
                                outp[s * 128 : (s + 1) * 128, :], ost[:]
                            )

                # ---- emission ----
                # Startup: stream x/cos/sin in REVERSE n-block order; after
                # block n arrives, project k/q0/q1 for it and immediately
                # emit head-0 (and head-1 for n>=1) scores for the c-chunks
                # that only need blocks >= n (ascending c within each batch
                # so exp(h,low c) lands first). This keeps ACT fed from ~5us.
                def ckpt(label):
                    PHASELOG.append((label, nc.next_id()))

                nc.sync.dma_start(wk_all[:], wkT[:, :])
                pts = {h: [None] * NCH for h in range(4)}
                mix_cm = tc.tile_pool(name="mix", bufs=2, space="PSUM")
                pools["mix"] = mix_cm.__enter__()
                xpool_cm = tc.tile_pool(name="xp", bufs=2)
                xpool = xpool_cm.__enter__()
                for n in (3, 2, 1, 0):
                    ckpt(f"nloop{n}")
                    nc.sync.dma_start(
                        cs_all[:, n * 1536 : (n + 1) * 1536],
                        csT[:, n * 1536 : (n + 1) * 1536],
                    )
                    xb = xpool.tile([128, 8 * 512], BF16, tag="xb", name=f"xb{n}")
                    nc.sync.dma_start(
                        xb[:, 0 : 4 * 512], xTi[:, n * 4096 : n * 4096 + 2048]
                    )
                    nc.sync.dma_start(
                        xb[:, 4 * 512 : 8 * 512],
                        xTi[:, n * 4096 + 2048 : (n + 1) * 4096],
                    )
                    if n == 3:
                        nc.sync.dma_start(wq_all[:], wqT[:, :])
                        nc.sync.dma_start(wvg_all[:], wvgT[:, :])
                        nc.sync.dma_start(dmask_sb[:], dmask[:, :])
                        make_identity(nc, ident_sb[:])
                    emit_k_proj(n, xb, qb_on_act=(n == 3))
                    cast_k8(k8_sb, kT_sb, n)
                    emit_q_proj(0, n, xb, qb_on_act=(n == 3))
                    cast_q8(q8_sb[0], qp01, n, 0)
                    cast_q8(q8_sb[1], qp01, n, 32)
                    pts[0][4 * n] = emit_scores_fp8(0, 4 * n, q8_sb[0])
                    pts[0][4 * n + 1] = emit_scores_fp8(0, 4 * n + 1, q8_sb[0])
                    emit_q_proj(1, n, xb, qb_on_act=(n == 3))
                    pts[0][4 * n + 2] = emit_scores_fp8(0, 4 * n + 2, q8_sb[0])
                    pts[0][4 * n + 3] = emit_scores_fp8(0, 4 * n + 3, q8_sb[0])
                    emit_vg(n, xb)
                    if n >= 1:
                        for c in range(4 * n, 4 * n + 4):
                            pts[1][c] = emit_scores_fp8(1, c, q8_sb[1])
                    if n >= 2:
                        for c in range(4 * n, 4 * n + 4):
                            pts[2][c] = emit_scores_fp8(2, c, q8_sb[2])
                xpool_cm.__exit__(None, None, None)
                late_cm = tc.tile_pool(name="late", bufs=1)
                late = late_cm.__enter__()
                wo_sb.append(late.tile([128, HM], BF16, tag="wo0", name="wo0"))
                wo_sb.append(late.tile([128, HM], BF16, tag="wo1", name="wo1"))
                for j in range(2):
                    nc.sync.dma_start(wo_sb[j][:], woT[j * 128 : (j + 1) * 128, :])

                mix_cm.__exit__(None, None, None)
                acc_cm = tc.tile_pool(name="acc", bufs=2, space="PSUM")
                pools["acc"] = acc_cm.__enter__()

                # Phase A: finish h1's wide chunks, then drain h0+h1 P@V
                # (their exps are already queued on ACT).
                ckpt("A-sc1")
                # gate: G = 1 + 0.5*(tanh(a/2) + tanh(b/2)); one batched tanh
                nc.scalar.activation(tg_all[:], gsb_all[:], AF.Tanh, scale=0.5)
                for s0 in range(NCH):
                    n0, sub0 = divmod(s0, 4)
                    o0 = n0 * 32 + sub0 * 8
                    gs = gpool.tile([128, 4], FP32, tag="gs", name="gs")
                    nc.gpsimd.tensor_tensor(
                        gs[:], tg_all[:, o0 : o0 + 4], tg_all[:, o0 + 4 : o0 + 8],
                        op=ALU.add,
                    )
                    nc.gpsimd.tensor_scalar(
                        g_sb[s0][:], gs[:], 0.5, 1.0, op0=ALU.mult, op1=ALU.add
                    )
                for c in range(4):
                    pts[1][c] = emit_scores_fp8(1, c, q8_sb[1])
                for c in range(NCH):
                    ckpt(f"A-sq{c}")
                    emit_sq_pair([(0, c, pts[0])])
                pts[2][0] = emit_scores_fp8(2, 0, q8_sb[2])
                pts[2][1] = emit_scores_fp8(2, 1, q8_sb[2])
                for c in range(NCH):
                    emit_sq_pair([(1, c, pts[1])])

                # Phase B: h2/h3 scores stream at full rate (exp supply for
                # ACT), both sq streams 2 chunks behind; h3's sq carries the
                # output projection so that work spreads across the phase.
                for c in range(18):
                    if 2 <= c <= 7:
                        ckpt(f"B{c}")
                        pts[2][c] = emit_scores_fp8(2, c, q8_sb[2])
                    if c <= 15:
                        pts[3][c] = emit_scores_fp8(3, c, q8_sb[3])
                    if 2 <= c <= 17:
                        emit_sq_pair([(2, c - 2, pts[2])])
                        emit_sq_pair([(3, c - 2, pts[3])])
                acc_cm.__exit__(None, None, None)
                late_cm.__exit__(None, None, None)

    _split_multi_waits(nc)
    return nc


_NC = None


def _get_nc() -> bass.Bass:
    global _NC
    if _NC is None:
        _NC = _build_program()
    return _NC


def _shard_inputs(
    hidden_states, Wq, Wk, Wv, Wo, Wkc, Wg_slc, Wg_swa
) -> list[dict[str, np.ndarray]]:
    bf16 = ml_dtypes.bfloat16
    f32 = np.float32

    # RoPE tables (bf16, feature-major, duplicated across two 64-row head
    # blocks). The rotation's half-swap is fused into the device multiply's
    # input AP, so the sin table here is stored half-SWAPPED and signed:
    # sinP[d] = +sin for d<32 (pairs with src d+32... device reads
    # sin[src-partition]), -sin for d>=32. Since sin/cos rows repeat with
    # period 32, the swap is a sign flip on the upper half.
    inv = 1.0 / (THETA ** (np.arange(0, D, 2, dtype=np.float64) / D))
    freqs = np.arange(S, dtype=np.float64)[:, None] * inv  # [S, 32]
    cos32 = np.cos(freqs).T  # [32, S]
    sin32 = np.sin(freqs).T
    # cos repeats every 32 rows (valid for both the pair-interleaved q
    # layout and the k layout). The sin tables are stored half-SWAPPED and
    # signed so the device's shifted multiply reads the right value at the
    # SOURCE partition: sinQ pairs rows p <-> p+64, sinK pairs p <-> p+32.
    cos2 = np.concatenate([cos32] * 4, axis=0).astype(bf16)  # [128, S]
    sinQ = np.concatenate([sin32, sin32, -sin32, -sin32], axis=0).astype(bf16)
    sinK = np.concatenate([sin32, -sin32, sin32, -sin32], axis=0).astype(bf16)
    csT = np.concatenate(
        [
            np.concatenate(
                [
                    cos2[:, n * 512 : (n + 1) * 512],
                    sinQ[:, n * 512 : (n + 1) * 512],
                    sinK[:, n * 512 : (n + 1) * 512],
                ],
                axis=1,
            )
            for n in range(4)
        ],
        axis=1,
    )
    csT = np.ascontiguousarray(csT)

    # pt[kv_i, sq_j] is valid iff kv <= sq, i.e. i <= j: upper triangular
    dmask = np.triu(np.ones((128, 128), dtype=f32)).astype(bf16)

    def interleave(w):
        """[1024, width] -> [128, 8*width] with hm-chunk-major columns so
        the whole weight loads in one contiguous DMA."""
        width = w.shape[1]
        return np.ascontiguousarray(
            w.reshape(8, 128, width).transpose(1, 0, 2).reshape(128, 8 * width)
        )

    in_maps = []
    for core in range(NCORES):
        b, g = divmod(core, 4)
        # xTi[p, n, kk, j] = x.T[kk*128+p, n*512+j]
        xTc = (
            np.ascontiguousarray(hidden_states[b].T)
            .astype(bf16)
            .reshape(8, 128, 4, 512)
            .transpose(1, 2, 0, 3)
            .reshape(128, 4 * 8 * 512)
        )
        xTc = np.ascontiguousarray(xTc)
        # pair-interleave each m-block's output features: rows become
        # [hA d0:32 | hB d0:32 | hA d32:64 | hB d32:64] for heads (2m, 2m+1)
        wq_g = Wq[g * 256 : (g + 1) * 256, :].reshape(2, 2, 2, 32, HM)
        # [m, head, half, 32, HM] -> [m, half, head, 32, HM]
        wq_g = np.ascontiguousarray(wq_g.transpose(0, 2, 1, 3, 4)).reshape(256, HM)
        wqTc = interleave(np.ascontiguousarray(wq_g.T).astype(bf16))
        wkTc = interleave(
            np.ascontiguousarray(Wk[g * 64 : (g + 1) * 64, :].T).astype(bf16)
        )
        wvg = np.concatenate(
            [
                Wv[g * 64 : (g + 1) * 64, :].T,
                Wg_slc[g * 4 : (g + 1) * 4, :].T,
                Wg_swa[g * 4 : (g + 1) * 4, :].T,
            ],
            axis=1,
        )  # [1024, 72]
        wvgc = interleave(np.ascontiguousarray(wvg).astype(bf16))
        woTc = np.ascontiguousarray(Wo[:, g * 256 : (g + 1) * 256].T).astype(bf16)
        in_maps.append(
            {
                "xTi": xTc,
                "wqT": wqTc,
                "wkT": wkTc,
                "wvgT": wvgc,
                "woT": woTc,
                "csT": csT,
                "dmask": dmask,
            }
        )
    return in_maps


def run(inputs: dict, trace: bool = False):
    """Run the SPMD kernel; returns (output [B,S,HM] f32, BassKernelResults)."""
    nc = _get_nc()
    in_maps = _shard_inputs(**inputs)
    res = run_bass_kernel_spmd(
        nc, in_maps, core_ids=list(range(NCORES)), trace=trace
    )
    out = np.zeros((B, S, HM), np.float32)
    for core in range(NCORES):
        b = core // 4
        out[b] += res.results[core]["outp"].astype(np.float32)
    return out, res


def kernel(**inputs) -> np.ndarray:
    out, _ = run(inputs)
    return out
